# revision 85
# baseline (speedup 1.0000x reference)
"""Bass/Trainium2 kernel for nn_BilinearPairedLayer.

Math (per batch b):
  xl = concat([x, shift_down(x,1), shift_up(x,1)], -1)      # [N, 192]
  xr = concat([x, shift_up(x,1), shift_down(x,1)], -1)
  hl = relu(xl @ W_l.T + b_l)                               # [N, 128]
  hr = relu(xr @ W_r.T + b_r)
  out[i,j,k] = sum_g (hl @ W_bil[k])[i,g] * hr[j,g] + b_bil[k]   # [N, N, 2]

Sharding: data-parallel over B — core c computes batch b=c (B=8, 8 cores).

Key design points:
  - the 8 MiB fp32 per-core output is stored as bf16 on device (4 MiB,
    halves the HBM drain) in two contiguous k-planes [NOUT, N, N]; the
    host interleaves k, casts to fp32, and adds b_bil. rel-err budget is
    2e-2; measured ~4.6e-3. Contiguous bf16 evictions are the fastest
    PSUM->SBUF path (fp16 or a (j,k)-interleaved layout cost 1.2-2.7x
    on ACT/DVE, which are the steady-state bottleneck next to HBM).
  - all matmul operands bf16; x arrives as x2 [128, 1028] with
    partitions 0-63 = x^T shifted -1, 64-127 = +1 (plus zero guard
    cols): the two shifted feature chunks contract in ONE K=128 matmul
    and the unshifted chunk is a K=64 matmul on a column-offset view.
  - inputs split across both HWDGE rings so the first chain's gates
    land early: sync ring carries bias (gates evictions), the W_l/W_r
    chunk pack, then w_bil; the scalar ring carries the first 260 x2
    cols (head_x, gates the A-chain) then x2 in two chunks.
  - schedule: minimal A-chain to the first 256-col tile; then all h/t
    chunks nearly back-to-back (dense PE keeps the HAM duty up and
    leaves no production arrows later); then a pure out stream of full
    512 KiB row DMAs. The final row's DMAs are halved for a short tail
    receipt.
  - evictions: out k-planes alternate ACT/DVE per tile; hl relu on ACT,
    hr relu on DVE, t0/t1 copies split DVE/ACT (gpsimd cannot read
    PSUM; DMA cannot touch PSUM at all).
  - out DMA triggers alternate Sync (HWDGE) and Pool (SWDGE) so the
    Scalar engine never spends ~0.6us/trigger instead of evicting.
  - a warm spinner of 128-col matmuls bridges the input window and
    keep-warm fillers (on a dedicated PSUM bank, so their WAR deps
    never couple to real evictions) hold the PE_HAM duty at 8/8
    (2.4 GHz) through the drain.
"""

import numpy as np

B, N, NIN = 8, 1024, 64
H = 128
NOUT = 2
NCH = 512   # matmul free-dim chunk (one PSUM bank of fp32)
GD = 2      # zero guard columns on each side of x2
HPW = 260   # x2 columns carried in the head pack (covers j<256 chunks)
NWARM = 30  # warm-spinner matmuls (128-col) bridging the input window

_cached = {}


def _build():
    import concourse.bacc as bacc
    import concourse.mybir as mybir
    import concourse.tile as tile

    f32 = mybir.dt.float32
    f16 = mybir.dt.float16
    bf16 = mybir.dt.bfloat16
    AF = mybir.ActivationFunctionType
    ALU = mybir.AluOpType

    nc = bacc.Bacc("TRN2", target_bir_lowering=False, debug=False, num_devices=8)

    # x2: partitions 0-63 = x^T(col-1), partitions 64-127 = x^T(col+1)
    # (an on-device sb2sb shift-copy of the bottom half measured slower:
    # its receipt gates every K=128 h-chunk)
    x2_d = nc.dram_tensor("x_2", [2 * NIN, N + 2 * GD], bf16, kind="ExternalInput").ap()
    # head_x: first HPW cols of x2 (feeds the A-chain h chunks);
    # head_w: w2, 3 slots of [128,128] flattened
    # w2 slots: 0 = [l_c1; l_c2], 1 = [r_c2; r_c1], 2 = [l_c0 | r_c0]
    hpx_d = nc.dram_tensor("head_x", [2 * NIN, HPW], bf16, kind="ExternalInput").ap()
    hpw_d = nc.dram_tensor("head_w", [2 * NIN, 3 * H], bf16, kind="ExternalInput").ap()
    wb_d = nc.dram_tensor("w_bil", [H, NOUT * H], bf16, kind="ExternalInput").ap()
    bias_d = nc.dram_tensor("bias_all", [128, 4], f32, kind="ExternalInput").ap()
    # two k-planes, contiguous j per row: evictions write contiguous
    # bf16 (fp16 or a (j,k)-interleaved layout cost 1.2-2.7x on ACT/DVE);
    # the host interleaves k during the fp32 cast and adds b_bil.
    out_d = nc.dram_tensor("out", [NOUT, N, N], bf16, kind="ExternalOutput").ap()
    # row-block view: [k, 8 blocks, 128 rows, 1024 j]
    out_v = out_d.rearrange("k (t p) n -> k t p n", p=128)

    with tile.TileContext(nc) as tc:
        with (
            tc.tile_pool(name="const", bufs=1) as const,
            tc.tile_pool(name="ps", bufs=7, space="PSUM") as ps,
            tc.tile_pool(name="fps", bufs=1, space="PSUM") as fps,
            tc.tile_pool(name="ob", bufs=8) as ob,
        ):
            # ---- input DMAs: tiny bias first (its receipt gates every
            # eviction), then the weight packs on the sync ring; head_x
            # + x2 chunks concurrently on the scalar ring
            bias = const.tile([128, 4], f32)
            nc.sync.dma_start(out=bias, in_=bias_d)
            hpw = const.tile([2 * NIN, 3 * H], bf16)
            nc.sync.dma_start(out=hpw, in_=hpw_d)
            wbC = const.tile([H, 2 * H], bf16)
            nc.sync.dma_start(out=wbC, in_=wb_d)
            hpx = const.tile([2 * NIN, HPW], bf16)
            nc.scalar.dma_start(out=hpx, in_=hpx_d)
            x2 = const.tile([2 * NIN, N + 2 * GD], bf16)
            XSP = 516
            nc.scalar.dma_start(out=x2[:, 0:XSP], in_=x2_d[:, 0:XSP])
            nc.scalar.dma_start(out=x2[:, XSP:], in_=x2_d[:, XSP:])

            def w2sl(s, p0=0, p1=2 * NIN):
                return hpw[p0:p1, s * H : (s + 1) * H]

            bl_s = bias[:, 0:1]
            br_s = bias[:, 1:2]
            bb_s = bias[:, 2:4]
            wb0 = wbC[:, 0:H]
            wb1 = wbC[:, H : 2 * H]

            # ---- PE warm spinner: ramp the HAM duty limit during the
            # input-DMA window (bf16 matmuls count as PE-busy). Small
            # 128-col matmuls so at most ~0.2us of spinner work can
            # delay the real chain once the head pack lands.
            warm = const.tile([128, 256], bf16)
            nc.vector.memset(warm, 0.0)
            # dummy ACT ops on a scratch tile: pull the lazy ACT table load
            # to the front without adding deps on `warm`
            actscratch = const.tile([1, 4], f32)
            nc.scalar.activation(actscratch[0:1, 0:2], warm[0:1, 0:2], AF.Relu)
            nc.scalar.activation(actscratch[0:1, 2:4], warm[0:1, 0:2], AF.Identity)
            for _ in range(NWARM):
                wps = ps.tile([128, NCH], f32, tag="ps")
                nc.tensor.matmul(
                    wps[:, 0:128], warm[:, 0:128], warm[:, 0:128],
                    start=True, stop=True, skip_group_check=True,
                )

            hlT = const.tile([H, N], bf16)
            hrT = const.tile([H, N], bf16)
            tT0 = const.tile([H, N], bf16)
            tT1 = const.tile([H, N], bf16)

            def h_chunk(dst, sl, bias_ap, j0, w, on_dve=False, xs=None):
                # sl 0: hl — shifted pair [l_c1; l_c2] @ x2 + l_c0 @ x(j)
                # sl 1: hr — shifted pair [r_c2; r_c1] @ x2 + r_c0 @ x(j)
                xs = x2 if xs is None else xs
                ph = ps.tile([128, NCH], f32, tag="ps")
                nc.tensor.matmul(
                    ph[:, 0:w], w2sl(sl), xs[:, GD + j0 : GD + j0 + w],
                    start=True, stop=False,
                )
                if sl == 0:
                    # unshifted x(j) = top half of x2 at col+1
                    nc.tensor.matmul(
                        ph[:, 0:w], w2sl(2, 0, NIN),
                        xs[0:NIN, GD + j0 + 1 : GD + j0 + 1 + w],
                        start=False, stop=True,
                    )
                else:
                    # unshifted x(j) = bottom half of x2 at col-1
                    nc.tensor.matmul(
                        ph[:, 0:w], w2sl(2, NIN, 2 * NIN),
                        xs[NIN : 2 * NIN, GD + j0 - 1 : GD + j0 - 1 + w],
                        start=False, stop=True,
                    )
                if on_dve:
                    nc.vector.tensor_scalar(
                        out=dst[:, j0 : j0 + w], in0=ph[:, 0:w],
                        scalar1=bias_ap, scalar2=0.0,
                        op0=ALU.add, op1=ALU.max,
                    )
                else:
                    nc.scalar.activation(
                        dst[:, j0 : j0 + w], ph[:, 0:w], AF.Relu,
                        bias=bias_ap, scale=1.0,
                    )

            def t_chunk(wb, tT, i0, w, on_act=False):
                pt = ps.tile([128, NCH], f32, tag="ps")
                nc.tensor.matmul(
                    pt[:, 0:w], wb, hlT[:, i0 : i0 + w], start=True, stop=True
                )
                if on_act:
                    # ACTIVATE IDENTITY(+bias) runs a faster uop program
                    # than ACTIVATE COPY on the scalar engine
                    nc.scalar.activation(
                        tT[:, i0 : i0 + w], pt[:, 0:w], AF.Identity,
                        bias=0.0, scale=1.0,
                    )
                else:
                    nc.vector.tensor_copy(tT[:, i0 : i0 + w], pt[:, 0:w])

            _dmaq = [0]

            def out_dma(dst_v, src):
                # triggers alternate Sync (HWDGE) / Pool (SWDGE): the
                # Scalar engine must not trigger (a ~0.6us DMA_DIRECT2D
                # per trigger would steal eviction bandwidth). The first
                # tile's two DMAs both ride the sync ring — SWDGE's ~1us
                # first-byte latency would sit on the drain's opening.
                _dmaq[0] += 1
                if _dmaq[0] <= 2:
                    eng = nc.sync
                else:
                    # 1:2 Pool:Sync — a SWDGE trigger costs the Pool engine
                    # ~0.75us, which paces the deferred tail at 1:1
                    eng = nc.gpsimd if _dmaq[0] % 3 == 0 else nc.sync
                eng.dma_start(out=dst_v, in_=src)

            _otile_n = [0]

            def out_evict(dst, po_sl, k):
                # plain fp32->fp16 copies (b_bil is added on the host —
                # a bias-from-PTR tensor_scalar costs ~1.6x); alternate
                # which engine takes k0/k1 per tile for DVE/ACT balance
                on_dve = (k == 0) ^ (_otile_n[0] % 2 == 1)
                if on_dve:
                    nc.vector.tensor_copy(dst, po_sl)
                else:
                    # IDENTITY+bias: faster scalar-engine table program
                    # than the COPY opcode
                    nc.scalar.activation(dst, po_sl, AF.Identity, bias=0.0, scale=1.0)

            _defq = []

            def flush_deferred():
                for dst_v, src in _defq:
                    out_dma(dst_v, src)
                del _defq[:]

            def out_tile(iblk, j0, w, last=False, defer=False):
                # one (iblk, j-range) tile: 2 matmuls (k=0,1); contiguous
                # psum->fp16 evictions; one DMA per k-plane
                _otile_n[0] += 1
                otile = ob.tile([128, 4 * NCH], bf16, tag="ob")
                for k, tT in ((0, tT0), (1, tT1)):
                    po = ps.tile([128, NCH], f32, tag="ps")
                    nc.tensor.matmul(
                        po[:, 0:w],
                        tT[:, iblk * 128 : (iblk + 1) * 128],
                        hrT[:, j0 : j0 + w],
                        start=True, stop=True,
                    )
                    out_evict(otile[:, 2 * k * NCH : 2 * k * NCH + w], po[:, 0:w], k)
                for k in (0, 1):
                    dst_v = out_v[k][iblk][:, j0 : j0 + w]
                    src = otile[:, 2 * k * NCH : 2 * k * NCH + w]
                    if last:
                        # halve the final DMAs so the tail receipt is short
                        q = w // 2
                        for qi in range(2):
                            _defq.append((dst_v[:, qi * q : (qi + 1) * q],
                                          src[:, qi * q : (qi + 1) * q]))
                    else:
                        _defq.append((dst_v, src))
                    if not defer:
                        flush_deferred()

            def out_row(iblk, last=False, defer=False):
                # full row (j 0..1024, both k): 4 matmuls sharing the two
                # ldweights, one 256 KiB DMA per k-plane
                _otile_n[0] += 1
                otile = ob.tile([128, 4 * NCH], bf16, tag="ob")
                for k, tT in ((0, tT0), (1, tT1)):
                    for jh in (0, 1):
                        po = ps.tile([128, NCH], f32, tag="ps")
                        nc.tensor.matmul(
                            po[:, :],
                            tT[:, iblk * 128 : (iblk + 1) * 128],
                            hrT[:, jh * NCH : (jh + 1) * NCH],
                            start=True, stop=True,
                        )
                        out_evict(
                            otile[:, (2 * k + jh) * NCH : (2 * k + jh + 1) * NCH],
                            po[:, :], k,
                        )
                for k in (0, 1):
                    src = otile[:, 2 * k * NCH : 2 * (k + 1) * NCH]
                    if last:
                        # halve the final DMAs so the tail receipt is short
                        for qi in range(2):
                            _defq.append((out_v[k][iblk][:, qi * NCH : (qi + 1) * NCH],
                                          src[:, qi * NCH : (qi + 1) * NCH]))
                    else:
                        _defq.append((out_v[k][iblk], src))
                    if not defer:
                        flush_deferred()

            def filler(n=1, w=256):
                # keep-warm matmuls: hold the PE_HAM duty at 8/8 through
                # slots where real PE work is below the DMA cadence.
                # Dedicated PSUM banks (fps pool) so a filler's WAR dep
                # never couples to a real tile's eviction — that would
                # block the in-order PE queue.
                for _ in range(n):
                    wf = fps.tile([128, NCH], f32, tag="fps")
                    nc.tensor.matmul(
                        wf[:, 0:w], warm[:, 0:128], warm[:, 0:w],
                        start=True, stop=True, skip_group_check=True,
                    )

            # ---- emission order: a minimal 128-col chain from the head
            # pack opens the output stream; h/t chunks are woven between
            # out tiles so the PE never stalls long; once hr is complete
            # the remaining rows drain as full 512 KiB DMAs.
            # A: minimal chain from the head pack opens the output stream
            Q = 256
            h_chunk(hlT, 0, bl_s, 0, 128, xs=hpx)
            h_chunk(hrT, 1, br_s, 0, Q, on_dve=True, xs=hpx)
            filler(1, 128)
            t_chunk(wb0, tT0, 0, 128)
            t_chunk(wb1, tT1, 0, 128, on_act=True)
            filler(1, 128)
            out_tile(0, 0, Q)
            # B: remaining h/t mostly back-to-back (dense PE; the A tiles
            # and two plug tiles keep the drain fed) so phase C is a pure
            # out stream with no production arrows left
            h_chunk(hlT, 0, bl_s, 128, 384)         # x2 landed by now
            h_chunk(hrT, 1, br_s, Q, Q, on_dve=True)
            filler(1, 128)
            out_tile(0, Q, Q)
            t_chunk(wb0, tT0, 128, 384)
            t_chunk(wb1, tT1, 128, 384, on_act=True)
            filler(1, 128)
            out_tile(1, 0, NCH)
            out_tile(2, 0, NCH)
            h_chunk(hlT, 0, bl_s, NCH, NCH)
            out_tile(3, 0, NCH)
            h_chunk(hrT, 1, br_s, NCH, NCH, on_dve=True)
            filler(1, 128)
            t_chunk(wb0, tT0, NCH, NCH)
            t_chunk(wb1, tT1, NCH, NCH, on_act=True)
            # C: pure out stream. Matmuls+evictions for ALL remaining
            # items run first (they finish at warm clock before the HAM
            # power demote, buffered in the 8 ob tiles); the DMA triggers
            # follow, so the tail is purely DMA-bound even at 4/8.
            # Tile (0,jh1) depends only on t_a + hr_c, so it fills the
            # PE idle gap while row 4 waits on t_c's eviction.
            out_tile(0, NCH, NCH, defer=True)
            out_row(4)
            out_row(5)
            out_row(6, defer=True)
            out_tile(2, NCH, NCH, defer=True)
            out_tile(3, NCH, NCH, defer=True)
            out_tile(1, NCH, NCH, defer=True)
            out_row(7, last=True, defer=True)
            flush_deferred()

    nc.finalize()
    return nc


def make_in_maps(x_l, W_l, b_l, W_r, b_r, W_bil, b_bil):
    # host-side layout:
    #   x2 [128, N+4]: rows 0-63 = x^T shifted -1 col, rows 64-127 = +1 col
    #   w2 [128, 3, H]: slot0=[l_c1;l_c2], slot1=[r_c2;r_c1], slot2=[l_c0|r_c0]
    #   w_bil [H, 2H] h-major, bias_all [128,4] = [b_l | b_r | b_bil bcast]
    import ml_dtypes

    bf = ml_dtypes.bfloat16

    def w_chunks(W):
        # -> [3, NIN, H] lhsT per chunk
        return np.asarray(W, np.float32).reshape(H, 3, NIN).transpose(1, 2, 0)

    x_l = np.asarray(x_l, np.float32)
    xt = np.zeros((B, NIN, N + 2 * GD), np.float32)
    xt[:, :, GD : GD + N] = x_l.transpose(0, 2, 1)
    x2 = np.zeros((B, 2 * NIN, N + 2 * GD), bf)
    x2[:, 0:NIN, 1:] = xt[:, :, :-1].astype(bf)   # row p = x^T at col-1
    x2[:, NIN:, :-1] = xt[:, :, 1:].astype(bf)    # row p = x^T at col+1

    lc = w_chunks(W_l)
    rc = w_chunks(W_r)
    w2 = np.zeros((2 * NIN, 3, H), np.float32)
    w2[0:NIN, 0] = lc[1]
    w2[NIN:, 0] = lc[2]
    w2[0:NIN, 1] = rc[2]
    w2[NIN:, 1] = rc[1]
    w2[0:NIN, 2] = lc[0]
    w2[NIN:, 2] = rc[0]

    bias_all = np.zeros((128, 4), np.float32)
    bias_all[:, 0] = np.asarray(b_l, np.float32)
    bias_all[:, 1] = np.asarray(b_r, np.float32)
    bias_all[:, 2:4] = np.asarray(b_bil, np.float32)[None, :]

    w2b = np.ascontiguousarray(w2).astype(bf).reshape(2 * NIN, 3 * H)
    wbb = np.ascontiguousarray(
        np.asarray(W_bil, np.float32).transpose(1, 0, 2).reshape(H, NOUT * H)
    ).astype(bf)
    return [
        {
            "x_2": np.ascontiguousarray(x2[c]),
            "head_x": np.ascontiguousarray(x2[c][:, 0:HPW]),
            "head_w": w2b,
            "w_bil": wbb,
            "bias_all": bias_all,
        }
        for c in range(B)
    ]


def kernel(x_l, W_l, b_l, W_r, b_r, W_bil, b_bil):
    from concourse import bass_utils

    if "nc" not in _cached:
        _cached["nc"] = _build()
    nc = _cached["nc"]

    in_maps = make_in_maps(x_l, W_l, b_l, W_r, b_r, W_bil, b_bil)
    res = bass_utils.run_bass_kernel_spmd(nc, in_maps, core_ids=list(range(B)))
    # device emits [NOUT, N, N] fp16 k-planes without b_bil; the host
    # interleaves k, casts to fp32, and adds the bias
    bb = np.asarray(b_bil, np.float32)
    return np.stack(
        [res.results[c]["out"].transpose(1, 2, 0).astype(np.float32) + bb
         for c in range(B)],
        axis=0,
    )
